# revision 25
# baseline (speedup 1.0000x reference)
"""Pairwise Euclidean distance for TRN2 (8 cores, SPMD): fp8 DoubleRow gram.

out[i,j] = ||mapping[i] - mapping[j]|| via d2 = sq_i + sq_j - 2 g. The
device computes only the gram matrix g, in fp8 e4m3 with DoubleRow matmuls
(2 weights/cell -> one K=256 matmul per [128,512] output tile), quantized
to uint8 by a runtime-calibrated affine (Cauchy-Schwarz bound +-sq_max, so
no saturation for any input; constants enter via a tiny [128,2] tensor read
as ACT Relu scale/bias APs and DVE tensor_scalar APs). The host dequants
via a 256-entry LUT and finishes sqrt(sq_i + sq_j - 2 g) in numpy; sq comes
from the same fp8-rounded vectors, so the metric is self-consistent and the
diagonal is exactly 0 (set explicitly).

Symmetry: each core computes a wrap-around band of W=4096 columns shifted
+512 past its own 1024 rows (every unique pair once); the host mirrors
transposes and computes the few structurally-uncovered "antipodal ring"
[512,512] tiles itself (~1.3 GFLOP of f32 GEMMs).

Error budget (measured end-to-end on hw): rel_absmax 1.087e-2 vs the 2e-2
gate (1.09e-2 / 1.21e-2 numpy-simulated on the axon- and cpu-generated
seed-0 datasets) — fp8 input rounding dominates; u8 gram quantization adds
~3e-3. The f16 (non-DoubleRow) variant in kernel_f16_backup.py measures
2.71e-3 at ~27 us if more margin is ever needed.

Per-core device pipeline (measured 15.4-16.8 us loop-NEFF; breakdown:
PE+input floor 10.6 us, epilogue-paced 12.9 us, + output DMA ~= the
~14.7 us DMA roofline of 5.1 MB/core at ~358 GB/s):
  - inputs: mt [128,2,4096] fp8 (x^T packed k=ko*128+ki for DoubleRow),
    mo [128,2,512] fp8 (own rows 0-511; rows 512-1023 reuse the first 512
    mt columns), qc [128,2] f32.
  - per row-tile r (8): 4 psum chunks [128,1024] (4 bufs = 8 banks), 2
    DoubleRow matmuls per chunk (~155 ns each incl. weight loads);
    epilogue u8 = qs*psum + qt on ACT/DVE/DVE/ACT; output staged per
    row-PAIR [128, 2W] and drained by one 1 MB dma (dma issue is ~0.4 us
    each and serialized, so count matters more than bytes).
  - the For_i timing loop ping-pongs two input tile sets so iteration
    i+1's loads are not WAR-blocked on iteration i's last matmuls.
"""

import sys

try:
    import concourse.bass as _probe  # noqa: F401
except ImportError:
    sys.path.insert(0, "/opt/trn_rl_repo")

import numpy as np

import concourse.bacc as bacc
import concourse.mybir as mybir
from concourse import tile
from concourse.bass_utils import run_bass_kernel_spmd

N = 8192
D = 256
NCORES = 8
RPC = N // NCORES
RT = RPC // 128
SUB = 512
NH = 8
W = NH * SUB
SHIFT = 512
CHUNK = 1024
NCK = W // CHUNK

F32 = mybir.dt.float32
FP8 = mybir.dt.float8e4
U8 = mybir.dt.uint8
NP8 = mybir.dt.np(FP8)

ENGINES = ("act", "dve", "dve", "act")

# DoubleRow weight layout [ki, ko, col] contracts k = ko*128 + ki
# (verified bit-for-bit against numpy on hw)
PACK_BLOCKED = True
# emit each parity's input loads in the previous loop body (SP rail issues
# them ahead of that body's out-dmas)
PREFETCH = True


def _build_nc(repeats=1, loop_n=None, stage_bufs=6):
    nc = bacc.Bacc(None, target_bir_lowering=False)
    mt_d = nc.dram_tensor("mt", [128, 2, W], FP8, kind="ExternalInput")
    # only own rows 0-511: rows 512-1023 are the first 512 cols of mt
    mo_d = nc.dram_tensor("mo", [128, 2, RPC // 2], FP8, kind="ExternalInput")
    qc_d = nc.dram_tensor("qc", [128, 2], F32, kind="ExternalInput")
    out_d = nc.dram_tensor("out", [RPC, W], U8, kind="ExternalOutput")

    with tile.TileContext(nc) as tc:
        with (
            tc.tile_pool(name="big", bufs=1) as big,
            tc.tile_pool(name="stage", bufs=stage_bufs) as stage,
            tc.tile_pool(name="ps", bufs=4, space="PSUM") as psum,
        ):
            qc = big.tile([128, 2], F32, tag="qc")
            nc.sync.dma_start(qc[:], qc_d[:])

            def tiles(par):
                m8 = big.tile([128, 2, W], FP8, tag=f"m8{par}",
                              name=f"m8_{par}")
                mo8 = big.tile([128, 2, RPC // 2], FP8, tag=f"mo8{par}",
                               name=f"mo8_{par}")
                return m8, mo8

            def loads(ts):
                # emitted in the PREVIOUS body so the SP rail issues these
                # ahead of that body's out-dmas (whose producers finish
                # last); otherwise the next body's matmuls gate on this
                # body's epilogue tail
                m8, mo8 = ts
                nc.sync.dma_start(mo8[:], mo_d[:])
                for c0 in range(0, W, 2048):
                    nc.sync.dma_start(m8[:, :, c0:c0 + 2048],
                                      mt_d[:, :, c0:c0 + 2048])

            tls = (tiles(0), tiles(1))
            if loop_n is not None:
                assert loop_n % 2 == 0
                if PREFETCH:
                    loads(tls[0])
                with tc.For_i(0, loop_n // 2, 1):
                    for par in (0, 1):
                        if PREFETCH:
                            pre = (lambda p=1 - par: loads(tls[p]))
                        else:
                            loads(tls[par])
                            pre = None
                        _emit_body(nc, stage, psum, out_d, qc, tls[par], pre)
            else:
                loads(tls[0])
                for rep in range(repeats):
                    par = rep % 2
                    if rep + 1 < repeats:
                        pre = (lambda p=1 - par: loads(tls[p]))
                    else:
                        pre = None
                    _emit_body(nc, stage, psum, out_d, qc, tls[par], pre)

    nc.compile()
    return nc


def _emit_body(nc, stage, psum, out_d, qc, ts, prefetch):
    m8, mo8 = ts
    if prefetch is not None:
        prefetch()
    qs = qc[:, 0:1]
    qt = qc[:, 1:2]

    engines = ENGINES
    for rp in range(RT // 2):
        # stage a row-PAIR [128, 2W]: both 128-row blocks are contiguous in
        # out dram, so ONE 1 MB dma drains them (dma issue is ~0.4 us each
        # and serialized -- count, not bytes, is what costs)
        ot = stage.tile([128, 2 * W], U8, tag="ot")
        for half in range(2):
            r = 2 * rp + half
            if r < 4:
                lhs = mo8[:, :, r * 128:(r + 1) * 128]
            else:
                # own rows 512-1023 are device cols 0-511 of the mt window
                lhs = m8[:, :, (r - 4) * 128:(r - 3) * 128]
            for ck in range(NCK):
                ps = psum.tile([128, CHUNK], F32, tag="ps")
                j0 = ck * CHUNK
                nc.tensor.matmul(ps[:, 0:SUB], lhs, m8[:, :, j0:j0 + SUB],
                                 start=True, stop=True,
                                 perf_mode=mybir.MatmulPerfMode.DoubleRow)
                nc.tensor.matmul(ps[:, SUB:CHUNK], lhs,
                                 m8[:, :, j0 + SUB:j0 + CHUNK],
                                 start=True, stop=True,
                                 perf_mode=mybir.MatmulPerfMode.DoubleRow)
                o = ot[:, half * W + j0:half * W + j0 + CHUNK]
                if engines[ck] == "act":
                    nc.scalar.activation(o, ps[:],
                                         mybir.ActivationFunctionType.Relu,
                                         bias=qt, scale=qs)
                else:
                    nc.vector.tensor_scalar(o, ps[:], qs, qt,
                                            mybir.AluOpType.mult,
                                            mybir.AluOpType.add)
        dst = out_d[2 * rp * 128:(2 * rp + 2) * 128, :].rearrange(
            "(g p) w -> p g w", g=2)
        nc.sync.dma_start(dst, ot[:].rearrange("p (g w) -> p g w", g=2))


_NC_CACHE = None


def _get_nc():
    global _NC_CACHE
    if _NC_CACHE is None:
        _NC_CACHE = _build_nc()
    return _NC_CACHE


def _pack(xt8: np.ndarray) -> np.ndarray:
    # xt8: [256, cols] fp8 -> [128, 2, cols] in the DR weight layout
    if PACK_BLOCKED:
        return np.ascontiguousarray(
            xt8.reshape(2, 128, -1).transpose(1, 0, 2))
    return np.ascontiguousarray(xt8.reshape(128, 2, -1))


def _quant_consts(x8f: np.ndarray):
    sqm = float((x8f * x8f).sum(axis=1).max()) * 1.02
    qs = 254.5 / (2.0 * sqm)
    qt = 0.25 + sqm * qs
    return qs, qt


def make_in_maps(mapping: np.ndarray) -> list:
    x8 = mapping.astype(np.float32).astype(NP8)
    x8f = x8.astype(np.float32)
    qs, qt = _quant_consts(x8f)
    qc = np.empty((128, 2), dtype=np.float32)
    qc[:, 0] = qs
    qc[:, 1] = qt
    xt8 = np.ascontiguousarray(x8.T)  # [256, 8192]
    in_maps = []
    for c in range(NCORES):
        j0 = c * RPC + SHIFT
        cols = np.arange(j0, j0 + W) % N
        mtc = _pack(np.ascontiguousarray(xt8[:, cols]))
        moc = _pack(np.ascontiguousarray(
            xt8[:, c * RPC:c * RPC + RPC // 2]))
        in_maps.append({"mt": mtc, "mo": moc, "qc": qc})
    return in_maps


def _direct(a, b):
    return (b - a + a % 2 - 1) % 16 <= 7


def kernel(mapping: np.ndarray, **_kwargs) -> np.ndarray:
    mapping = np.asarray(mapping, dtype=np.float32)
    assert mapping.shape == (N, D)
    in_maps = make_in_maps(mapping)

    nc = _get_nc()
    res = run_bass_kernel_spmd(nc, in_maps, core_ids=list(range(NCORES)))

    x8f = mapping.astype(NP8).astype(np.float32)
    sq = np.einsum("ij,ij->i", x8f, x8f).astype(np.float32)
    qs, qt = _quant_consts(x8f)
    lut = (-2.0 * ((np.arange(256, dtype=np.float64) - qt) / qs)
           ).astype(np.float32)

    G = np.empty((N, N), dtype=np.float32)
    for c in range(NCORES):
        deq = lut[res.results[c]["out"]]
        r0 = c * RPC
        for h in range(NH):
            cb = (2 * c + 1 + h) % 16
            G[r0:r0 + RPC, cb * SUB:(cb + 1) * SUB] = \
                deq[:, h * SUB:(h + 1) * SUB]

    done = set()
    for a in range(16):
        for b in range(16):
            if _direct(a, b) or _direct(b, a) or (b, a) in done:
                continue
            t = x8f[a * SUB:(a + 1) * SUB] @ x8f[b * SUB:(b + 1) * SUB].T
            t *= -2.0
            G[a * SUB:(a + 1) * SUB, b * SUB:(b + 1) * SUB] = t
            if a != b:
                G[b * SUB:(b + 1) * SUB, a * SUB:(a + 1) * SUB] = t.T
            done.add((a, b))
    for a in range(16):
        for b in range(16):
            if not _direct(a, b) and _direct(b, a):
                G[a * SUB:(a + 1) * SUB, b * SUB:(b + 1) * SUB] = \
                    G[b * SUB:(b + 1) * SUB, a * SUB:(a + 1) * SUB].T

    G += sq[:, None]
    G += sq[None, :]
    np.clip(G, 0.0, None, out=G)
    np.sqrt(G, out=G)
    np.fill_diagonal(G, 0.0)
    return G


# revision 28
# speedup vs baseline: 1.4842x; 1.4842x over previous
"""Pairwise Euclidean distance for TRN2 (8 cores, SPMD): fp8 DoubleRow gram.

out[i,j] = ||mapping[i] - mapping[j]|| via d2 = sq_i + sq_j - 2 g. The
device computes only the gram matrix g, in fp8 e4m3 with DoubleRow matmuls
(2 weights/cell -> one K=256 matmul per [128,512] output tile), quantized
to uint8 by a runtime-calibrated affine (Cauchy-Schwarz bound +-sq_max, so
no saturation for any input; constants enter via a tiny [128,2] tensor read
as ACT Relu scale/bias APs and DVE tensor_scalar APs). The host dequants
via a 256-entry LUT and finishes sqrt(sq_i + sq_j - 2 g) in numpy; sq comes
from the same fp8-rounded vectors, so the metric is self-consistent and the
diagonal is exactly 0 (set explicitly).

Symmetry: each core computes a wrap-around band of W=4096 columns shifted
+512 past its own 1024 rows (every unique pair once); the host mirrors
transposes and computes the few structurally-uncovered "antipodal ring"
[512,512] tiles itself (~1.3 GFLOP of f32 GEMMs).

Error budget (measured end-to-end on hw): rel_absmax 1.087e-2 vs the 2e-2
gate (1.09e-2 / 1.21e-2 numpy-simulated on the axon- and cpu-generated
seed-0 datasets) — fp8 input rounding dominates; u8 gram quantization adds
~3e-3. The f16 (non-DoubleRow) variant in kernel_f16_backup.py measures
2.71e-3 at ~27 us if more margin is ever needed.

Per-core device pipeline (measured 15.4-16.8 us loop-NEFF; breakdown:
PE+input floor 10.6 us, epilogue-paced 12.9 us, + output DMA ~= the
~14.7 us DMA roofline of 5.1 MB/core at ~358 GB/s):
  - inputs: mt [128,2,4096] fp8 (x^T packed k=ko*128+ki for DoubleRow),
    mo [128,2,512] fp8 (own rows 0-511; rows 512-1023 reuse the first 512
    mt columns), qc [128,2] f32.
  - per row-tile r (8): 4 psum chunks [128,1024] (4 bufs = 8 banks), 2
    DoubleRow matmuls per chunk (~155 ns each incl. weight loads);
    epilogue u8 = qs*psum + qt on ACT/DVE/DVE/ACT; output staged per
    row-PAIR [128, 2W] and drained by one 1 MB dma (dma issue is ~0.4 us
    each and serialized, so count matters more than bytes).
  - the For_i timing loop ping-pongs two input tile sets so iteration
    i+1's loads are not WAR-blocked on iteration i's last matmuls, and
    each parity's loads are EMITTED in the previous body (PREFETCH) so the
    SP rail issues them ahead of that body's out-dmas — without this the
    next body's matmuls gate on this body's epilogue tail (drift-controlled
    A/B: 13.1-14.9 us prefetched vs 14.2-20.0 us and erratic without).
"""

import sys

try:
    import concourse.bass as _probe  # noqa: F401
except ImportError:
    sys.path.insert(0, "/opt/trn_rl_repo")

import numpy as np

import concourse.bacc as bacc
import concourse.mybir as mybir
from concourse import tile
from concourse.bass_utils import run_bass_kernel_spmd

N = 8192
D = 256
NCORES = 8
RPC = N // NCORES
RT = RPC // 128
SUB = 512
NH = 8
W = NH * SUB
SHIFT = 512
CHUNK = 1024
NCK = W // CHUNK

F32 = mybir.dt.float32
FP8 = mybir.dt.float8e4
U8 = mybir.dt.uint8
NP8 = mybir.dt.np(FP8)

ENGINES = ("act", "dve", "dve", "act")

# DoubleRow weight layout [ki, ko, col] contracts k = ko*128 + ki
# (verified bit-for-bit against numpy on hw)
PACK_BLOCKED = True
# emit each parity's input loads in the previous loop body (SP rail issues
# them ahead of that body's out-dmas)
PREFETCH = True
# which engine issues output dmas: "sync" (SP, HWDGE) or "gpsimd" (POOL,
# SWDGE) — POOL is otherwise idle, taking outs off the SP load rail
OUT_ENGINE = "sync"


def _build_nc(repeats=1, loop_n=None, stage_bufs=6):
    nc = bacc.Bacc(None, target_bir_lowering=False)
    mt_d = nc.dram_tensor("mt", [128, 2, W], FP8, kind="ExternalInput")
    # only own rows 0-511: rows 512-1023 are the first 512 cols of mt
    mo_d = nc.dram_tensor("mo", [128, 2, RPC // 2], FP8, kind="ExternalInput")
    qc_d = nc.dram_tensor("qc", [128, 2], F32, kind="ExternalInput")
    out_d = nc.dram_tensor("out", [RPC, W], U8, kind="ExternalOutput")

    with tile.TileContext(nc) as tc:
        with (
            tc.tile_pool(name="big", bufs=1) as big,
            tc.tile_pool(name="stage", bufs=stage_bufs) as stage,
            tc.tile_pool(name="ps", bufs=4, space="PSUM") as psum,
        ):
            qc = big.tile([128, 2], F32, tag="qc")
            nc.sync.dma_start(qc[:], qc_d[:])

            def tiles(par):
                m8 = big.tile([128, 2, W], FP8, tag=f"m8{par}",
                              name=f"m8_{par}")
                mo8 = big.tile([128, 2, RPC // 2], FP8, tag=f"mo8{par}",
                               name=f"mo8_{par}")
                return m8, mo8

            def loads(ts):
                # emitted in the PREVIOUS body so the SP rail issues these
                # ahead of that body's out-dmas (whose producers finish
                # last); otherwise the next body's matmuls gate on this
                # body's epilogue tail
                m8, mo8 = ts
                nc.sync.dma_start(mo8[:], mo_d[:])
                for c0 in range(0, W, 2048):
                    nc.sync.dma_start(m8[:, :, c0:c0 + 2048],
                                      mt_d[:, :, c0:c0 + 2048])

            tls = (tiles(0), tiles(1))
            if loop_n is not None:
                assert loop_n % 2 == 0
                if PREFETCH:
                    loads(tls[0])
                with tc.For_i(0, loop_n // 2, 1):
                    for par in (0, 1):
                        if PREFETCH:
                            pre = (lambda p=1 - par: loads(tls[p]))
                        else:
                            loads(tls[par])
                            pre = None
                        _emit_body(nc, stage, psum, out_d, qc, tls[par], pre)
            else:
                loads(tls[0])
                for rep in range(repeats):
                    par = rep % 2
                    if rep + 1 < repeats:
                        pre = (lambda p=1 - par: loads(tls[p]))
                    else:
                        pre = None
                    _emit_body(nc, stage, psum, out_d, qc, tls[par], pre)

    nc.compile()
    return nc


def _emit_body(nc, stage, psum, out_d, qc, ts, prefetch):
    m8, mo8 = ts
    if prefetch is not None:
        prefetch()
    qs = qc[:, 0:1]
    qt = qc[:, 1:2]

    engines = ENGINES
    for rp in range(RT // 2):
        # stage a row-PAIR [128, 2W]: both 128-row blocks are contiguous in
        # out dram, so ONE 1 MB dma drains them (dma issue is ~0.4 us each
        # and serialized -- count, not bytes, is what costs)
        ot = stage.tile([128, 2 * W], U8, tag="ot")
        for half in range(2):
            r = 2 * rp + half
            if r < 4:
                lhs = mo8[:, :, r * 128:(r + 1) * 128]
            else:
                # own rows 512-1023 are device cols 0-511 of the mt window
                lhs = m8[:, :, (r - 4) * 128:(r - 3) * 128]
            for ck in range(NCK):
                ps = psum.tile([128, CHUNK], F32, tag="ps")
                j0 = ck * CHUNK
                nc.tensor.matmul(ps[:, 0:SUB], lhs, m8[:, :, j0:j0 + SUB],
                                 start=True, stop=True,
                                 perf_mode=mybir.MatmulPerfMode.DoubleRow)
                nc.tensor.matmul(ps[:, SUB:CHUNK], lhs,
                                 m8[:, :, j0 + SUB:j0 + CHUNK],
                                 start=True, stop=True,
                                 perf_mode=mybir.MatmulPerfMode.DoubleRow)
                o = ot[:, half * W + j0:half * W + j0 + CHUNK]
                if engines[ck] == "act":
                    nc.scalar.activation(o, ps[:],
                                         mybir.ActivationFunctionType.Relu,
                                         bias=qt, scale=qs)
                else:
                    nc.vector.tensor_scalar(o, ps[:], qs, qt,
                                            mybir.AluOpType.mult,
                                            mybir.AluOpType.add)
        dst = out_d[2 * rp * 128:(2 * rp + 2) * 128, :].rearrange(
            "(g p) w -> p g w", g=2)
        eng = nc.gpsimd if OUT_ENGINE == "gpsimd" else nc.sync
        eng.dma_start(dst, ot[:].rearrange("p (g w) -> p g w", g=2))


_NC_CACHE = None


def _get_nc():
    global _NC_CACHE
    if _NC_CACHE is None:
        _NC_CACHE = _build_nc()
    return _NC_CACHE


def _pack(xt8: np.ndarray) -> np.ndarray:
    # xt8: [256, cols] fp8 -> [128, 2, cols] in the DR weight layout
    if PACK_BLOCKED:
        return np.ascontiguousarray(
            xt8.reshape(2, 128, -1).transpose(1, 0, 2))
    return np.ascontiguousarray(xt8.reshape(128, 2, -1))


def _quant_consts(x8f: np.ndarray):
    sqm = float((x8f * x8f).sum(axis=1).max()) * 1.02
    qs = 254.5 / (2.0 * sqm)
    qt = 0.25 + sqm * qs
    return qs, qt


def make_in_maps(mapping: np.ndarray) -> list:
    x8 = mapping.astype(np.float32).astype(NP8)
    x8f = x8.astype(np.float32)
    qs, qt = _quant_consts(x8f)
    qc = np.empty((128, 2), dtype=np.float32)
    qc[:, 0] = qs
    qc[:, 1] = qt
    xt8 = np.ascontiguousarray(x8.T)  # [256, 8192]
    in_maps = []
    for c in range(NCORES):
        j0 = c * RPC + SHIFT
        cols = np.arange(j0, j0 + W) % N
        mtc = _pack(np.ascontiguousarray(xt8[:, cols]))
        moc = _pack(np.ascontiguousarray(
            xt8[:, c * RPC:c * RPC + RPC // 2]))
        in_maps.append({"mt": mtc, "mo": moc, "qc": qc})
    return in_maps


def _direct(a, b):
    return (b - a + a % 2 - 1) % 16 <= 7


def kernel(mapping: np.ndarray, **_kwargs) -> np.ndarray:
    mapping = np.asarray(mapping, dtype=np.float32)
    assert mapping.shape == (N, D)
    in_maps = make_in_maps(mapping)

    nc = _get_nc()
    res = run_bass_kernel_spmd(nc, in_maps, core_ids=list(range(NCORES)))

    x8f = mapping.astype(NP8).astype(np.float32)
    sq = np.einsum("ij,ij->i", x8f, x8f).astype(np.float32)
    qs, qt = _quant_consts(x8f)
    lut = (-2.0 * ((np.arange(256, dtype=np.float64) - qt) / qs)
           ).astype(np.float32)

    G = np.empty((N, N), dtype=np.float32)
    for c in range(NCORES):
        deq = lut[res.results[c]["out"]]
        r0 = c * RPC
        for h in range(NH):
            cb = (2 * c + 1 + h) % 16
            G[r0:r0 + RPC, cb * SUB:(cb + 1) * SUB] = \
                deq[:, h * SUB:(h + 1) * SUB]

    done = set()
    for a in range(16):
        for b in range(16):
            if _direct(a, b) or _direct(b, a) or (b, a) in done:
                continue
            t = x8f[a * SUB:(a + 1) * SUB] @ x8f[b * SUB:(b + 1) * SUB].T
            t *= -2.0
            G[a * SUB:(a + 1) * SUB, b * SUB:(b + 1) * SUB] = t
            if a != b:
                G[b * SUB:(b + 1) * SUB, a * SUB:(a + 1) * SUB] = t.T
            done.add((a, b))
    for a in range(16):
        for b in range(16):
            if not _direct(a, b) and _direct(b, a):
                G[a * SUB:(a + 1) * SUB, b * SUB:(b + 1) * SUB] = \
                    G[b * SUB:(b + 1) * SUB, a * SUB:(a + 1) * SUB].T

    G += sq[:, None]
    G += sq[None, :]
    np.clip(G, 0.0, None, out=G)
    np.sqrt(G, out=G)
    np.fill_diagonal(G, 0.0)
    return G
